# revision 11
# baseline (speedup 1.0000x reference)
"""LipschitzRNN Trainium2 kernel (v3: PSUM-resident recurrence, era output).

Math (per reference):
    bA = 0.5*exp(-bA_z^2)+0.5 ; bW likewise
    A = (1-bA)(MA+MA.T) + bA(MA-MA.T) - YA*I
    C = (1-bA)(MW+MW.T) + bW(MW-MW.T) - YW*I
    X_{t+1} = X_t + STEP*(A@X_t + tanh(C@X_t + by))   (column-state X: [n, bs])
    out[b, t, :] = X_t[:, b]

Device strategy (8-way batch data-parallel, b=32/core, no collectives).
The serial chain per step is ACT(tanh) -> 4 PE matmuls -> ACT.  The tanh
input W_t = C@X_t + by is accumulated in PSUM across steps; to keep every
W-matmul except STEP*C@tq_t off the chain (in-place PSUM writes must wait
for the previous tanh read of the same bank), W lives in two parity banks
updated with a 2-step formula:

  W_{t+2} = W_t + [2*STEP*CA + STEP^2*CA@A]@xq_t
                + [STEP*C + STEP^2*CA]@tq_t + STEP*C@tq_{t+1}

X is also PSUM-resident, accumulated by PE (STEP*A@xq + STEP*I@tq); a
single DVE cast per step produces the fp16 copy xq (matmul moving
operand).  fp16 stationary rounding drift in W is killed by a refresh
every 64 steps (W_t := (C+STEP*CA)@xq_{t-1} + by + STEP*C@tq_{t-1}; the
`by` matmul is emitted first with start=True covering the full bank
footprint -- PSUM zero regions are coarser than one m-chunk).

Timing notes (measured): a PE matmul that blocks on a semaphore at the
queue head costs ~+150ns at wakeup; one whose wait was already satisfied
when reached issues back-to-back (27ns).  Filler matmuls (scr target)
at the end of each iteration keep PE busy past the tq-semaphore arrival
so the chain burst issues hot.  tanh with a 16-bit output has a fixed
~260ns cost (vs ~145 for f32 out) -- bf16 out is used since the matmul
needs a 16-bit moving operand anyway.

Output: xq is written into per-16-step "era" tiles [128, 16, 64] f16 and
DMA'd raw ([n, b] layout, fp16); the final [b, t, n] fp32 layout is pure
glue done on the host during unsharding.
"""

import numpy as np

N = 256
BS = 256
TMAX = 512
STEP = 0.01
YA = 0.001
YW = 0.001
NCORES = 8
BLOC = BS // NCORES  # 32
NSTEPS = TMAX - 1    # 511
REFRESH = 64
ERA = 16
NERAS = (NSTEPS + ERA - 1) // ERA  # 32 (last era has 15 steps)

LAST_RESULT = None  # BassKernelResults of the most recent run (for test harness)


def _build():
    from concourse import bacc, tile
    import concourse.mybir as mybir

    F32 = mybir.dt.float32
    F16 = mybir.dt.float16
    BF16 = mybir.dt.bfloat16
    AF = mybir.ActivationFunctionType

    nc = bacc.Bacc("TRN2", target_bir_lowering=False, debug=False,
                   num_devices=NCORES)

    # constants arrive as 3 bundled tensors (one dma_start each; a
    # dma_start costs ~585ns serialized on the sync queue).
    # BB (bf16): wc1 blocks 0:512, wctq 512:1024, wi 1024:1152
    # BF (f16): wcxq 0:512, wa1 512:1024, wrf 1024:1536, by2 1536:1664,
    #           sel 1664:1728, idt 1728:1856
    # B32 (f32): idf 0:128, x0sb 128:192, w0sb 192:256, w1sb 256:320
    BB = nc.dram_tensor("BB", [128, 1152], BF16, kind="ExternalInput")
    BF = nc.dram_tensor("BF", [128, 1856], F16, kind="ExternalInput")
    B32 = nc.dram_tensor("B32", [128, 320], F32, kind="ExternalInput")
    # raw xq dump: OUTE[e, p, s, col] = fp16 X_{16e+s+1}[chunk(col)*128+p, b(col)]
    # (partition dim leads so the DMA AP aligns with the SBUF tile)
    OUTE = nc.dram_tensor("OUTE", [NERAS, 128, ERA, 2 * BLOC], F16,
                          kind="ExternalOutput")

    B = BLOC

    with tile.TileContext(nc) as tc:
        with (
            tc.tile_pool(name="consts", bufs=1) as consts,
            tc.tile_pool(name="xqpool", bufs=2) as xqpool,
            tc.tile_pool(name="tqpool", bufs=3) as tqpool,
            tc.tile_pool(name="wev", bufs=1, space="PSUM") as wevp,
            tc.tile_pool(name="wod", bufs=1, space="PSUM") as wodp,
            tc.tile_pool(name="xps", bufs=1, space="PSUM") as xpsp,
        ):
            # ---- constants (3 bundled DMAs; state bundle first so the
            # init matmuls + tanh_0 + ACT table load overlap the big
            # weight transfers) ----
            b32 = consts.tile([128, 320], F32, tag="b32")
            nc.sync.dma_start(b32[:], B32[:, :])
            bb = consts.tile([128, 1152], BF16, tag="bb")
            nc.sync.dma_start(bb[:], BB[:, :])
            bf = consts.tile([128, 1856], F16, tag="bf")
            nc.sync.dma_start(bf[:], BF[:, :])

            def blk(tile, off, k, m):
                i = k * 2 + m
                return tile[:, off + i * 128: off + (i + 1) * 128]

            wc1 = [[blk(bb, 0, k, m) for m in range(2)] for k in range(2)]
            wctq = [[blk(bb, 512, k, m) for m in range(2)] for k in range(2)]
            wi = bb[:, 1024:1152]
            wcxq = [[blk(bf, 0, k, m) for m in range(2)] for k in range(2)]
            wa1 = [[blk(bf, 512, k, m) for m in range(2)] for k in range(2)]
            wrf = [[blk(bf, 1024, k, m) for m in range(2)] for k in range(2)]
            by2 = bf[:, 1536:1664]
            sel = bf[:, 1664:1728]
            idt = bf[:, 1728:1856]
            idf = b32[:, 0:128]
            x0sb = b32[:, 128:192]
            w0sb = b32[:, 192:256]
            w1sb = b32[:, 256:320]

            prf32 = consts.tile([128, 2 * B], F32, tag="prf32")
            prbf = consts.tile([128, 2 * B], BF16, tag="prbf")

            # ---- persistent PSUM state ----
            wev = wevp.tile([128, 2 * B], F32, tag="wev")   # W_t, even t
            wod = wodp.tile([128, 2 * B], F32, tag="wod")   # W_t, odd t
            xp = xpsp.tile([128, 2 * B], F32, tag="xp")     # X_t

            # init: W_0 (even), W_1 (odd, host-computed), X_0 via identity mm
            nc.tensor.matmul(wev[:], idf, w0sb, start=True, stop=True)
            nc.tensor.matmul(wod[:], idf, w1sb, start=True, stop=True)
            nc.tensor.matmul(xp[:], idf, x0sb, start=True, stop=True)

            # xq_0 / tq_0
            xq0 = consts.tile([128, 2 * B], F16, tag="xq0")
            nc.vector.tensor_copy(xq0[:], x0sb)
            tq = {}
            tq[0] = tqpool.tile([128, 2 * B], BF16, tag="tq", name="tq")
            nc.scalar.activation(tq[0][:], wev[:], AF.Tanh, bias=0.0, scale=1.0)

            era_tiles = {}

            def xq_ref(t):
                if t == 0:
                    return xq0[:, 0:2 * B]
                e, s = (t - 1) // ERA, (t - 1) % ERA
                return era_tiles[e][:, s, :]

            def wbank(t):
                return wev if t % 2 == 0 else wod

            def acc_mm(out_region, lhsT, rhs, start=False):
                nc.tensor.matmul(out_region, lhsT, rhs, start=start,
                                 stop=True, skip_group_check=True)

            # ---- main loop: iter t produces W_t (t>=2), X_t, tanh_t, xq_t ----
            for t in range(1, NSTEPS + 1):
                is_rf = (t % REFRESH) in (0, 1)
                nx_rf = ((t + 1) % REFRESH) in (0, 1)
                wb = wbank(t)
                tqp = tq[t - 1]
                xqp = xq_ref(t - 1)

                # chain matmuls completing W_t (and refresh prologue)
                if 2 <= t <= NSTEPS - 1:
                    if is_rf:
                        # W := by (one start=True mm covering the full
                        # [*,0:64] footprint), then accumulate (C+STEP*CA)@xq
                        acc_mm(wb[:, 0:2 * B], by2, sel, start=True)
                        for m in range(2):
                            r = wb[:, m * B:(m + 1) * B]
                            acc_mm(r, wrf[0][m], xqp[:, 0:B])
                            acc_mm(r, wrf[1][m], xqp[:, B:2 * B])
                    for k in range(2):
                        for m in range(2):
                            acc_mm(wb[:, m * B:(m + 1) * B], wc1[k][m],
                                   tqp[:, k * B:(k + 1) * B])

                # X_t accumulation
                for m in range(2):
                    r = xp[:, m * B:(m + 1) * B]
                    acc_mm(r, wi, tqp[:, m * B:(m + 1) * B])
                    acc_mm(r, wa1[0][m], xqp[:, 0:B])
                    acc_mm(r, wa1[1][m], xqp[:, B:2 * B])

                # off-chain 2-step terms for W_{t+1}
                if t + 1 <= NSTEPS - 1 and not nx_rf:
                    wn = wbank(t + 1)
                    for m in range(2):
                        r = wn[:, m * B:(m + 1) * B]
                        acc_mm(r, wcxq[0][m], xqp[:, 0:B])
                        acc_mm(r, wcxq[1][m], xqp[:, B:2 * B])
                        acc_mm(r, wctq[0][m], tqp[:, 0:B])
                        acc_mm(r, wctq[1][m], tqp[:, B:2 * B])

                # tanh_t
                if t <= NSTEPS - 1:
                    tq[t] = tqpool.tile([128, 2 * B], BF16, tag="tq", name="tq")
                    nc.scalar.activation(tq[t][:], wb[:], AF.Tanh,
                                         bias=0.0, scale=1.0)
                tq.pop(t - 2, None)

                # ACT-cost probes (off-chain, idle ACT time): f32-out vs
                # bf16-out tanh on a const input, for trace calibration
                if t % 16 == 5:
                    nc.scalar.activation(prf32[:], x0sb, AF.Tanh,
                                         bias=0.0, scale=1.0)
                elif t % 16 == 13:
                    nc.scalar.activation(prbf[:], x0sb, AF.Tanh,
                                         bias=0.0, scale=1.0)

                # xq_t (DVE, off-chain); era tile also IS the output staging
                e, s = (t - 1) // ERA, (t - 1) % ERA
                if s == 0:
                    era_tiles[e] = xqpool.tile([128, ERA, 2 * B], F16,
                                               tag="era", name="era")
                nc.vector.tensor_copy(era_tiles[e][:, s, :], xp[:])
                if s == ERA - 1:
                    nc.sync.dma_start(OUTE[e, :, :, :], era_tiles[e][:, :, :])
                    era_tiles.pop(e - 2, None)

            # tail: era 31 has 15 steps
            nc.sync.dma_start(OUTE[NERAS - 1, :, 0:ERA - 1, :],
                              era_tiles[NERAS - 1][:, 0:ERA - 1, :])

    nc.compile()
    return nc


def kernel(X0, MA, MW, bA_z, bW_z, by_w):
    global LAST_RESULT
    from concourse.bass_utils import run_bass_kernel_spmd
    import ml_dtypes

    X0 = np.asarray(X0, dtype=np.float32)
    MA = np.asarray(MA, dtype=np.float32)
    MW = np.asarray(MW, dtype=np.float32)
    bA_z = np.asarray(bA_z, dtype=np.float32)
    bW_z = np.asarray(bW_z, dtype=np.float32)
    by = np.asarray(by_w, dtype=np.float32)

    bA = np.float32(0.5) * np.exp(-bA_z[0, 0] * bA_z[0, 0]) + np.float32(0.5)
    bW = np.float32(0.5) * np.exp(-bW_z[0, 0] * bW_z[0, 0]) + np.float32(0.5)
    I = np.eye(N, dtype=np.float32)
    A = (1 - bA) * (MA + MA.T) + bA * (MA - MA.T) - np.float32(YA) * I
    C = (1 - bA) * (MW + MW.T) + bW * (MW - MW.T) - np.float32(YW) * I

    A64, C64 = A.astype(np.float64), C.astype(np.float64)
    CA = C64 @ A64
    CAA = CA @ A64

    def f16T(M):
        return np.ascontiguousarray(M.T.astype(np.float32)).astype(np.float16)

    def bf16T(M):
        return np.ascontiguousarray(M.T.astype(np.float32)).astype(ml_dtypes.bfloat16)

    def blocks(M):  # [256,256] -> [128, 512] in (k,m) block order
        return np.hstack([M[128 * k:128 * (k + 1), 128 * m:128 * (m + 1)]
                          for k in range(2) for m in range(2)])

    WC1 = bf16T(STEP * C64)
    WCXQ = f16T(2 * STEP * CA + STEP * STEP * CAA)
    WCTQ = bf16T(STEP * C64 + STEP * STEP * CA)
    WA1 = f16T(STEP * A64)
    WRF = f16T(C64 + STEP * CA)
    BY2 = np.zeros((128, 128), dtype=np.float16)
    BY2[0, :] = by[0:128, 0].astype(np.float16)
    BY2[1, :] = by[128:256, 0].astype(np.float16)
    SEL = np.zeros((128, 2 * BLOC), dtype=np.float16)
    SEL[0, 0:BLOC] = 1
    SEL[1, BLOC:2 * BLOC] = 1
    WI = (np.eye(128, dtype=np.float32) * np.float32(STEP)).astype(ml_dtypes.bfloat16)
    IDT = np.eye(128, dtype=np.float16)
    IDF = np.eye(128, dtype=np.float32)
    BB = np.hstack([blocks(WC1), blocks(WCTQ), WI]).astype(ml_dtypes.bfloat16)
    BFb = np.hstack([blocks(WCXQ), blocks(WA1), blocks(WRF), BY2, SEL,
                     IDT]).astype(np.float16)

    CA32 = CA.astype(np.float32)

    def pack(M):  # [256, b] -> [128, 2b] chunk-major
        return np.concatenate([M[0:128, :], M[128:256, :]], axis=1)

    in_maps = []
    for i in range(NCORES):
        X0c = np.ascontiguousarray(X0[i * BLOC:(i + 1) * BLOC, :].T)  # [256, 32]
        W0 = C @ X0c + by
        xq0 = X0c.astype(np.float16).astype(np.float32)
        tq0 = np.tanh(W0).astype(ml_dtypes.bfloat16).astype(np.float32)
        W1 = W0 + np.float32(STEP) * (CA32 @ xq0 + C @ tq0)
        B32b = np.hstack([IDF, pack(X0c), pack(W0.astype(np.float32)),
                          pack(W1.astype(np.float32))]).astype(np.float32)
        in_maps.append({
            "BB": BB, "BF": BFb,
            "B32": np.ascontiguousarray(B32b),
        })

    nc = _build()
    res = run_bass_kernel_spmd(nc, in_maps, core_ids=list(range(NCORES)))
    LAST_RESULT = res

    # unshard + layout: OUTE[e, p, s, c*32+b] = X_{16e+s+1}[c*128+p, b]
    out = np.empty((BS, TMAX, N), dtype=np.float32)
    out[:, 0, :] = X0
    for i in range(NCORES):
        raw = np.asarray(res.results[i]["OUTE"]).astype(np.float32)
        # [e, p, s, cv] -> [t-1, c, b, p] -> [b, t, c*128+p]
        arr = raw.transpose(0, 2, 3, 1).reshape(NERAS * ERA, 2, BLOC, 128)[0:NSTEPS]
        out[i * BLOC:(i + 1) * BLOC, 1:TMAX, :] = (
            arr.transpose(2, 0, 1, 3).reshape(BLOC, NSTEPS, N))
    return out


if __name__ == "__main__":
    rng = np.random.default_rng(0)
    inputs = {
        "X0": rng.standard_normal((BS, N), dtype=np.float32),
        "MA": rng.standard_normal((N, N), dtype=np.float32) / 16,
        "MW": rng.standard_normal((N, N), dtype=np.float32) / 16,
        "bA_z": np.full((1, 1), 0.65, dtype=np.float32),
        "bW_z": np.full((1, 1), 0.65, dtype=np.float32),
        "by_w": rng.standard_normal((N, 1), dtype=np.float32) / 100,
    }
    out = kernel(**inputs)
    print("out", out.shape, out.dtype, np.abs(out).max())


# revision 12
# speedup vs baseline: 1.1626x; 1.1626x over previous
"""LipschitzRNN Trainium2 kernel (v3: PSUM-resident recurrence, era output).

Math (per reference):
    bA = 0.5*exp(-bA_z^2)+0.5 ; bW likewise
    A = (1-bA)(MA+MA.T) + bA(MA-MA.T) - YA*I
    C = (1-bA)(MW+MW.T) + bW(MW-MW.T) - YW*I
    X_{t+1} = X_t + STEP*(A@X_t + tanh(C@X_t + by))   (column-state X: [n, bs])
    out[b, t, :] = X_t[:, b]

Device strategy (8-way batch data-parallel, b=32/core, no collectives).
The serial chain per step is ACT(tanh) -> 4 PE matmuls -> ACT.  The tanh
input W_t = C@X_t + by is accumulated in PSUM across steps; to keep every
W-matmul except STEP*C@tq_t off the chain (in-place PSUM writes must wait
for the previous tanh read of the same bank), W lives in two parity banks
updated with a 2-step formula:

  W_{t+2} = W_t + [2*STEP*CA + STEP^2*CA@A]@xq_t
                + [STEP*C + STEP^2*CA]@tq_t + STEP*C@tq_{t+1}

X is also PSUM-resident, accumulated by PE (STEP*A@xq + STEP*I@tq); a
single DVE cast per step produces the fp16 copy xq (matmul moving
operand).  fp16 stationary rounding drift in W is killed by a refresh
every 64 steps (W_t := (C+STEP*CA)@xq_{t-1} + by + STEP*C@tq_{t-1}; the
`by` matmul is emitted first with start=True covering the full bank
footprint -- PSUM zero regions are coarser than one m-chunk).

Timing notes (measured): a PE matmul that blocks on a semaphore at the
queue head costs ~+150ns at wakeup; one whose wait was already satisfied
when reached issues back-to-back (27ns).  Filler matmuls (scr target)
at the end of each iteration keep PE busy past the tq-semaphore arrival
so the chain burst issues hot.  tanh with a 16-bit output has a fixed
~260ns cost (vs ~145 for f32 out) -- bf16 out is used since the matmul
needs a 16-bit moving operand anyway.

Output: xq is written into per-16-step "era" tiles [128, 16, 64] f16 and
DMA'd raw ([n, b] layout, fp16); the final [b, t, n] fp32 layout is pure
glue done on the host during unsharding.
"""

import numpy as np

N = 256
BS = 256
TMAX = 512
STEP = 0.01
YA = 0.001
YW = 0.001
NCORES = 8
BLOC = BS // NCORES  # 32
NSTEPS = TMAX - 1    # 511
REFRESH = 64
ERA = 16
NERAS = (NSTEPS + ERA - 1) // ERA  # 32 (last era has 15 steps)

LAST_RESULT = None  # BassKernelResults of the most recent run (for test harness)


def _build():
    from concourse import bacc, tile
    import concourse.mybir as mybir

    F32 = mybir.dt.float32
    F16 = mybir.dt.float16
    BF16 = mybir.dt.bfloat16
    AF = mybir.ActivationFunctionType

    nc = bacc.Bacc("TRN2", target_bir_lowering=False, debug=False,
                   num_devices=NCORES)

    # stationaries, all pre-transposed ([k, m] layout)
    WC1 = nc.dram_tensor("WC1", [N, N], BF16, kind="ExternalInput")   # (STEP*C).T
    WCXQ = nc.dram_tensor("WCXQ", [N, N], F16, kind="ExternalInput")  # (2*STEP*CA+STEP^2*CAA).T
    WCTQ = nc.dram_tensor("WCTQ", [N, N], BF16, kind="ExternalInput")  # (STEP*C+STEP^2*CA).T
    WA1 = nc.dram_tensor("WA1", [N, N], F16, kind="ExternalInput")    # (STEP*A).T
    WRF = nc.dram_tensor("WRF", [N, N], F16, kind="ExternalInput")    # (C+STEP*CA).T
    BY2 = nc.dram_tensor("BY2", [128, 128], F16, kind="ExternalInput")  # by in rows 0/1
    SEL = nc.dram_tensor("SEL", [128, 2 * BLOC], F16, kind="ExternalInput")  # chunk selector
    WI = nc.dram_tensor("WI", [128, 128], BF16, kind="ExternalInput")  # diag(bf16(STEP))
    IDT = nc.dram_tensor("IDT", [128, 128], F16, kind="ExternalInput")   # identity f16
    IDF = nc.dram_tensor("IDF", [128, 128], F32, kind="ExternalInput")   # identity f32
    X0SB = nc.dram_tensor("X0SB", [128, 2 * BLOC], F32, kind="ExternalInput")
    W0SB = nc.dram_tensor("W0SB", [128, 2 * BLOC], F32, kind="ExternalInput")
    W1SB = nc.dram_tensor("W1SB", [128, 2 * BLOC], F32, kind="ExternalInput")
    # raw xq dump: OUTE[e, p, s, col] = fp16 X_{16e+s+1}[chunk(col)*128+p, b(col)]
    # (partition dim leads so the DMA AP aligns with the SBUF tile)
    OUTE = nc.dram_tensor("OUTE", [NERAS, 128, ERA, 2 * BLOC], F16,
                          kind="ExternalOutput")

    B = BLOC

    with tile.TileContext(nc) as tc:
        with (
            tc.tile_pool(name="consts", bufs=1) as consts,
            tc.tile_pool(name="xqpool", bufs=2) as xqpool,
            tc.tile_pool(name="tqpool", bufs=3) as tqpool,
            tc.tile_pool(name="wev", bufs=1, space="PSUM") as wevp,
            tc.tile_pool(name="wod", bufs=1, space="PSUM") as wodp,
            tc.tile_pool(name="xps", bufs=1, space="PSUM") as xpsp,
        ):
            # ---- constants (state DMAs first so init mms + tanh_0 +
            # ACT table load overlap the weight transfers) ----
            x0sb_t = consts.tile([128, 2 * B], F32, tag="x0sb")
            nc.sync.dma_start(x0sb_t[:], X0SB[:, :])
            w0sb_t = consts.tile([128, 2 * B], F32, tag="w0sb")
            nc.sync.dma_start(w0sb_t[:], W0SB[:, :])
            w1sb_t = consts.tile([128, 2 * B], F32, tag="w1sb")
            nc.sync.dma_start(w1sb_t[:], W1SB[:, :])
            idf_t = consts.tile([128, 128], F32, tag="idf")
            nc.sync.dma_start(idf_t[:], IDF[:, :])

            def load4(dram, tag, dt=F16):
                ts = [[consts.tile([128, 128], dt, tag=f"{tag}{k}{m}", name=f"{tag}{k}{m}")
                       for m in range(2)] for k in range(2)]
                for k in range(2):
                    for m in range(2):
                        nc.sync.dma_start(
                            ts[k][m][:],
                            dram[128 * k:128 * (k + 1), 128 * m:128 * (m + 1)])
                return ts

            wc1b = load4(WC1, "wc1", BF16)
            wa1b = load4(WA1, "wa1")
            wi_t = consts.tile([128, 128], BF16, tag="wi")
            nc.sync.dma_start(wi_t[:], WI[:, :])
            wcxqb = load4(WCXQ, "wcxq")
            wctqb = load4(WCTQ, "wctq", BF16)
            wrfb = load4(WRF, "wrf")
            by2_t = consts.tile([128, 128], F16, tag="by2")
            nc.sync.dma_start(by2_t[:], BY2[:, :])
            sel_t = consts.tile([128, 2 * B], F16, tag="sel")
            nc.sync.dma_start(sel_t[:], SEL[:, :])
            idt_t = consts.tile([128, 128], F16, tag="idt")
            nc.sync.dma_start(idt_t[:], IDT[:, :])

            wc1 = [[wc1b[k][m][:] for m in range(2)] for k in range(2)]
            wcxq = [[wcxqb[k][m][:] for m in range(2)] for k in range(2)]
            wctq = [[wctqb[k][m][:] for m in range(2)] for k in range(2)]
            wa1 = [[wa1b[k][m][:] for m in range(2)] for k in range(2)]
            wrf = [[wrfb[k][m][:] for m in range(2)] for k in range(2)]
            wi = wi_t[:]
            by2 = by2_t[:]
            sel = sel_t[:]
            idt = idt_t[:]
            idf = idf_t[:]
            x0sb = x0sb_t[:]
            w0sb = w0sb_t[:]
            w1sb = w1sb_t[:]

            # ---- persistent PSUM state ----
            wev = wevp.tile([128, 2 * B], F32, tag="wev")   # W_t, even t
            wod = wodp.tile([128, 2 * B], F32, tag="wod")   # W_t, odd t
            xp = xpsp.tile([128, 2 * B], F32, tag="xp")     # X_t

            # init: W_0 (even), W_1 (odd, host-computed), X_0 via identity mm
            nc.tensor.matmul(wev[:], idf, w0sb, start=True, stop=True)
            nc.tensor.matmul(wod[:], idf, w1sb, start=True, stop=True)
            nc.tensor.matmul(xp[:], idf, x0sb, start=True, stop=True)

            # xq_0 / tq_0
            xq0 = consts.tile([128, 2 * B], F16, tag="xq0")
            nc.vector.tensor_copy(xq0[:], x0sb)
            tq = {}
            tq[0] = tqpool.tile([128, 2 * B], BF16, tag="tq", name="tq")
            nc.scalar.activation(tq[0][:], wev[:], AF.Tanh, bias=0.0, scale=1.0)

            era_tiles = {}

            def xq_ref(t):
                if t == 0:
                    return xq0[:, 0:2 * B]
                e, s = (t - 1) // ERA, (t - 1) % ERA
                return era_tiles[e][:, s, :]

            def wbank(t):
                return wev if t % 2 == 0 else wod

            def acc_mm(out_region, lhsT, rhs, start=False):
                nc.tensor.matmul(out_region, lhsT, rhs, start=start,
                                 stop=True, skip_group_check=True)

            # ---- main loop: iter t produces W_t (t>=2), X_t, tanh_t, xq_t ----
            for t in range(1, NSTEPS + 1):
                is_rf = (t % REFRESH) in (0, 1)
                nx_rf = ((t + 1) % REFRESH) in (0, 1)
                wb = wbank(t)
                tqp = tq[t - 1]
                xqp = xq_ref(t - 1)

                # chain matmuls completing W_t (and refresh prologue)
                if 2 <= t <= NSTEPS - 1:
                    if is_rf:
                        # W := by (one start=True mm covering the full
                        # [*,0:64] footprint), then accumulate (C+STEP*CA)@xq
                        acc_mm(wb[:, 0:2 * B], by2, sel, start=True)
                        for m in range(2):
                            r = wb[:, m * B:(m + 1) * B]
                            acc_mm(r, wrf[0][m], xqp[:, 0:B])
                            acc_mm(r, wrf[1][m], xqp[:, B:2 * B])
                    for k in range(2):
                        for m in range(2):
                            acc_mm(wb[:, m * B:(m + 1) * B], wc1[k][m],
                                   tqp[:, k * B:(k + 1) * B])

                # X_t accumulation
                for m in range(2):
                    r = xp[:, m * B:(m + 1) * B]
                    acc_mm(r, wi, tqp[:, m * B:(m + 1) * B])
                    acc_mm(r, wa1[0][m], xqp[:, 0:B])
                    acc_mm(r, wa1[1][m], xqp[:, B:2 * B])

                # off-chain 2-step terms for W_{t+1}
                if t + 1 <= NSTEPS - 1 and not nx_rf:
                    wn = wbank(t + 1)
                    for m in range(2):
                        r = wn[:, m * B:(m + 1) * B]
                        acc_mm(r, wcxq[0][m], xqp[:, 0:B])
                        acc_mm(r, wcxq[1][m], xqp[:, B:2 * B])
                        acc_mm(r, wctq[0][m], tqp[:, 0:B])
                        acc_mm(r, wctq[1][m], tqp[:, B:2 * B])

                # tanh_t
                if t <= NSTEPS - 1:
                    tq[t] = tqpool.tile([128, 2 * B], BF16, tag="tq", name="tq")
                    nc.scalar.activation(tq[t][:], wb[:], AF.Tanh,
                                         bias=0.0, scale=1.0)
                tq.pop(t - 2, None)

                # xq_t (DVE, off-chain); era tile also IS the output staging
                e, s = (t - 1) // ERA, (t - 1) % ERA
                if s == 0:
                    era_tiles[e] = xqpool.tile([128, ERA, 2 * B], F16,
                                               tag="era", name="era")
                nc.vector.tensor_copy(era_tiles[e][:, s, :], xp[:])
                if s == ERA - 1:
                    nc.sync.dma_start(OUTE[e, :, :, :], era_tiles[e][:, :, :])
                    era_tiles.pop(e - 2, None)

            # tail: era 31 has 15 steps
            nc.sync.dma_start(OUTE[NERAS - 1, :, 0:ERA - 1, :],
                              era_tiles[NERAS - 1][:, 0:ERA - 1, :])

    nc.compile()
    return nc


def kernel(X0, MA, MW, bA_z, bW_z, by_w):
    global LAST_RESULT
    from concourse.bass_utils import run_bass_kernel_spmd
    import ml_dtypes

    X0 = np.asarray(X0, dtype=np.float32)
    MA = np.asarray(MA, dtype=np.float32)
    MW = np.asarray(MW, dtype=np.float32)
    bA_z = np.asarray(bA_z, dtype=np.float32)
    bW_z = np.asarray(bW_z, dtype=np.float32)
    by = np.asarray(by_w, dtype=np.float32)

    bA = np.float32(0.5) * np.exp(-bA_z[0, 0] * bA_z[0, 0]) + np.float32(0.5)
    bW = np.float32(0.5) * np.exp(-bW_z[0, 0] * bW_z[0, 0]) + np.float32(0.5)
    I = np.eye(N, dtype=np.float32)
    A = (1 - bA) * (MA + MA.T) + bA * (MA - MA.T) - np.float32(YA) * I
    C = (1 - bA) * (MW + MW.T) + bW * (MW - MW.T) - np.float32(YW) * I

    A64, C64 = A.astype(np.float64), C.astype(np.float64)
    CA = C64 @ A64
    CAA = CA @ A64

    def f16T(M):
        return np.ascontiguousarray(M.T.astype(np.float32)).astype(np.float16)

    def bf16T(M):
        return np.ascontiguousarray(M.T.astype(np.float32)).astype(ml_dtypes.bfloat16)

    WC1 = bf16T(STEP * C64)
    WCXQ = f16T(2 * STEP * CA + STEP * STEP * CAA)
    WCTQ = bf16T(STEP * C64 + STEP * STEP * CA)
    WA1 = f16T(STEP * A64)
    WRF = f16T(C64 + STEP * CA)
    BY2 = np.zeros((128, 128), dtype=np.float16)
    BY2[0, :] = by[0:128, 0].astype(np.float16)
    BY2[1, :] = by[128:256, 0].astype(np.float16)
    SEL = np.zeros((128, 2 * BLOC), dtype=np.float16)
    SEL[0, 0:BLOC] = 1
    SEL[1, BLOC:2 * BLOC] = 1
    WI = (np.eye(128, dtype=np.float32) * np.float32(STEP)).astype(ml_dtypes.bfloat16)
    IDT = np.eye(128, dtype=np.float16)
    IDF = np.eye(128, dtype=np.float32)


    CA32 = CA.astype(np.float32)

    def pack(M):  # [256, b] -> [128, 2b] chunk-major
        return np.concatenate([M[0:128, :], M[128:256, :]], axis=1)

    in_maps = []
    for i in range(NCORES):
        X0c = np.ascontiguousarray(X0[i * BLOC:(i + 1) * BLOC, :].T)  # [256, 32]
        W0 = C @ X0c + by
        xq0 = X0c.astype(np.float16).astype(np.float32)
        tq0 = np.tanh(W0).astype(ml_dtypes.bfloat16).astype(np.float32)
        W1 = W0 + np.float32(STEP) * (CA32 @ xq0 + C @ tq0)
        in_maps.append({
            "WC1": WC1, "WCXQ": WCXQ, "WCTQ": WCTQ, "WA1": WA1, "WRF": WRF,
            "BY2": BY2, "SEL": SEL, "WI": WI, "IDT": IDT, "IDF": IDF,
            "X0SB": np.ascontiguousarray(pack(X0c)),
            "W0SB": np.ascontiguousarray(pack(W0.astype(np.float32))),
            "W1SB": np.ascontiguousarray(pack(W1.astype(np.float32))),
        })

    nc = _build()
    res = run_bass_kernel_spmd(nc, in_maps, core_ids=list(range(NCORES)))
    LAST_RESULT = res

    # unshard + layout: OUTE[e, p, s, c*32+b] = X_{16e+s+1}[c*128+p, b]
    out = np.empty((BS, TMAX, N), dtype=np.float32)
    out[:, 0, :] = X0
    for i in range(NCORES):
        raw = np.asarray(res.results[i]["OUTE"]).astype(np.float32)
        # [e, p, s, cv] -> [t-1, c, b, p] -> [b, t, c*128+p]
        arr = raw.transpose(0, 2, 3, 1).reshape(NERAS * ERA, 2, BLOC, 128)[0:NSTEPS]
        out[i * BLOC:(i + 1) * BLOC, 1:TMAX, :] = (
            arr.transpose(2, 0, 1, 3).reshape(BLOC, NSTEPS, N))
    return out


if __name__ == "__main__":
    rng = np.random.default_rng(0)
    inputs = {
        "X0": rng.standard_normal((BS, N), dtype=np.float32),
        "MA": rng.standard_normal((N, N), dtype=np.float32) / 16,
        "MW": rng.standard_normal((N, N), dtype=np.float32) / 16,
        "bA_z": np.full((1, 1), 0.65, dtype=np.float32),
        "bW_z": np.full((1, 1), 0.65, dtype=np.float32),
        "by_w": rng.standard_normal((N, 1), dtype=np.float32) / 100,
    }
    out = kernel(**inputs)
    print("out", out.shape, out.dtype, np.abs(out).max())
